# revision 1
# baseline (speedup 1.0000x reference)
"""KNN flow accumulation (AccFlow) Trainium2 kernel.

Problem: for each of Nq=16384 query points (3D), find k=3 nearest of
Nr=16384 ref points (Euclidean), take inverse-distance-weighted average
of the corresponding ref_flow vectors.

Sharding: queries split 2048/core across 8 NeuronCores; refs replicated.

Numerics: the reference (jax on the neuron backend) computes
    d2 = fl(fl(q2 + r2) - fl(2*(q@rT)));  dist = fl(sqrt(max(d2, 0)))
and the dataset contains near-coincident point pairs where d2 suffers
catastrophic fp32 cancellation, so the top-3 picks and the 1/d weights
are determined by the *specific rounding* of that exact formula.  All
pieces are reproduced bit-exactly on device:

  - 2qr: K=3 PE matmul with pre-doubled refs.  The PE's fp32 matmul is
    internally a bf16-decomposition that bit-matches XLA's q@rT (x2
    scaling commutes exactly with every rounding step; verified 0/2M
    mismatches vs jnp on this hardware).
  - r2 broadcast: K=1 ones-matmul through PSUM (bit-exact, verified),
    copied once into a resident [128, 16384] SBUF tile.
  - nd2 = fl(2qr - fl(q2+r2)) = -d2 via two wide DVE ops per group:
    X = fl(r2b + q2) (tensor_scalar, per-partition q2), then the
    in-place subtract fl(psum - X) (tensor_tensor, IEEE RN).
  - top-8 per query row: DVE `max` (sorted top-8 over the full
    16384-wide row in one instruction) + `max_index`.

The per-query candidate count is 8 >= k=3, so the exact top-3 (with the
reference's (dist, index) tie-breaking) always lies inside the top-8
except for >8-way degenerate-tie rows (probability ~0).  The final
rank/weight/gather epilogue runs as the *identical sequence of unjitted
jnp ops the reference itself executes* (sqrt, 1/(d+eps), normalize,
fancy-index gather, weighted sum) on the same backend, so the rounding
matches the reference bit-for-bit.

Performance note: this execution stack is dispatch-bound (~50-90us per
instruction regardless of size; indirect-DMA gathers likewise ~70us per
call), so the kernel minimizes instruction count above all: 8-matmul
PSUM groups, [128, 4096]-wide DVE ops, one `max`/`max_index` per
128-query block, outputs batched into two DMAs per core.
"""

import os
import sys

import numpy as np

for _p in ("/opt/trn_rl_repo", os.path.expanduser("~/.axon_site/_ro/trn_rl_repo")):
    if os.path.isdir(_p) and _p not in sys.path:
        sys.path.insert(0, _p)

import concourse.bacc as bacc
import concourse.mybir as mybir
from concourse.bass_utils import run_bass_kernel_spmd
from concourse.tile import TileContext

F32 = mybir.dt.float32
U32 = mybir.dt.uint32

N_CORES = 8
NQ = 16384
NR = 16384
K = 3
EPS = 1e-8

P = 128                    # queries per block (partition dim)
NQ_CORE = NQ // N_CORES    # 2048
NB = NQ_CORE // P          # 16 blocks per core
CH = 512                   # matmul free-dim chunk (one PSUM bank)
GRP = 4096                 # DVE group width (8 PSUM banks)


def build_nc(reps=1):
    nc = bacc.Bacc(None, target_bir_lowering=False)

    # consts rows 0-2: [2*rT | qT]; row 3 (stored at partition 32): [r2 | 0]
    consts = nc.declare_dram_parameter(
        "consts", [4, NR + NQ_CORE], F32, isOutput=False
    )
    # q2 in block-major layout: q2b[p, b] = |q[b*128 + p]|^2
    q2b = nc.declare_dram_parameter("q2b", [P, NB], F32, isOutput=False)
    v8o = nc.declare_dram_parameter("v8o", [NQ_CORE, 8], F32, isOutput=True)
    i8o = nc.declare_dram_parameter("i8o", [NQ_CORE, 8], U32, isOutput=True)

    with TileContext(nc) as tc:
        with (
            tc.tile_pool(name="const", bufs=1) as const_pool,
            tc.tile_pool(name="score", bufs=1) as s_pool,
            tc.tile_pool(name="small", bufs=1) as sm_pool,
        ):
            # matmul operands must share a base partition in {0,32,64}:
            # 2rT|qT at partitions 0-2, r2 at partition 32.
            c_sb = const_pool.tile([33, NR + NQ_CORE], F32, tag="consts")
            nc.sync.dma_start(out=c_sb[:3, :], in_=consts[:3, :])
            nc.sync.dma_start(out=c_sb[32:33, :NR], in_=consts[3:4, :NR])
            rT_sb = c_sb[:3, :NR]          # 2*rT
            r2row = c_sb[32:33, :NR]       # r2
            q2_all = const_pool.tile([P, NB], F32, tag="q2b")
            nc.sync.dma_start(out=q2_all[:], in_=q2b[:, :])
            ones = const_pool.tile([33, P], F32, tag="ones")
            nc.vector.memset(ones[32:33, :], 1.0)

            r2b = const_pool.tile([P, NR], F32, tag="r2b")
            vall = sm_pool.tile([P, NB * 8], F32, tag="vall")
            iall = sm_pool.tile([P, NB * 8], U32, tag="iall")

            # prologue: broadcast r2 across partitions, once
            with tc.tile_pool(name="ppsum", bufs=2, space="PSUM") as pp:
                for c0 in range(0, NR, CH):
                    psr = pp.tile([P, CH], F32, tag="psr")
                    nc.tensor.matmul(
                        out=psr[:], lhsT=ones[32:33, :],
                        rhs=r2row[:, c0:c0 + CH],
                        start=True, stop=True,
                    )
                    nc.scalar.copy(out=r2b[:, c0:c0 + CH], in_=psr[:])

            with tc.tile_pool(name="psum", bufs=1, space="PSUM") as psum_pool:
                for b in list(range(NB)) * reps:
                    qT3 = c_sb[:3, NR + b * P:NR + (b + 1) * P]   # [3,128]
                    q2_sb = q2_all[:, b:b + 1]
                    s_sb = s_pool.tile([P, NR], F32, tag="s")

                    for g0 in range(0, NR, GRP):
                        psA = psum_pool.tile([P, GRP], F32, tag="psA")
                        for j in range(GRP // CH):
                            c0 = g0 + j * CH
                            nc.tensor.matmul(
                                out=psA[:, j * CH:(j + 1) * CH],
                                lhsT=qT3, rhs=rT_sb[:, c0:c0 + CH],
                                start=True, stop=True,
                            )
                        sl = s_sb[:, g0:g0 + GRP]
                        # X = fl(r2 + q2)
                        nc.vector.tensor_scalar_add(
                            sl, r2b[:, g0:g0 + GRP], q2_sb
                        )
                        # nd2 = fl(2qr - X) = -d2, bit-exact (in place)
                        nc.vector.tensor_tensor(
                            out=sl, in0=psA[:], in1=sl,
                            op=mybir.AluOpType.subtract,
                        )

                    nc.vector.max(out=vall[:, b * 8:(b + 1) * 8], in_=s_sb[:])
                    nc.vector.max_index(
                        out=iall[:, b * 8:(b + 1) * 8],
                        in_max=vall[:, b * 8:(b + 1) * 8],
                        in_values=s_sb[:],
                    )

            # batched outputs: v8o[b*128+p, j] = vall[p, b*8+j]
            nc.sync.dma_start(
                out=v8o[:, :].rearrange("(b p) j -> p b j", p=P),
                in_=vall[:].rearrange("p (b j) -> p b j", j=8),
            )
            nc.sync.dma_start(
                out=i8o[:, :].rearrange("(b p) j -> p b j", p=P),
                in_=iall[:].rearrange("p (b j) -> p b j", j=8),
            )

    nc.finalize()
    return nc


_NC_CACHE = None


def _get_nc():
    global _NC_CACHE
    if _NC_CACHE is None:
        _NC_CACHE = build_nc()
    return _NC_CACHE


def _prep_core_inputs(q, r):
    qT = q.T.astype(np.float32)
    q2 = (q * q).sum(axis=1, dtype=np.float32).astype(np.float32)
    r2 = (r * r).sum(axis=1, dtype=np.float32).astype(np.float32)
    return qT, q2, r2


def kernel(query_points, ref_points, ref_flow, k):
    assert int(k) == K
    q = np.ascontiguousarray(np.asarray(query_points, dtype=np.float32))
    r = np.ascontiguousarray(np.asarray(ref_points, dtype=np.float32))
    f = np.ascontiguousarray(np.asarray(ref_flow, dtype=np.float32))
    assert q.shape == (NQ, 3) and r.shape == (NR, 3)

    qT, q2, r2 = _prep_core_inputs(q, r)
    r2T = np.empty((4, NR), dtype=np.float32)
    r2T[:3] = (2.0 * r.T).astype(np.float32)
    r2T[3] = r2

    nc = _get_nc()
    in_maps = []
    for c in range(N_CORES):
        s = slice(c * NQ_CORE, (c + 1) * NQ_CORE)
        qpart = np.zeros((4, NQ_CORE), dtype=np.float32)
        qpart[:3] = qT[:, s]
        consts = np.concatenate([r2T, qpart], axis=1)
        q2c = q2[s].reshape(NB, P).T
        in_maps.append({
            "consts": np.ascontiguousarray(consts),
            "q2b": np.ascontiguousarray(q2c),
        })

    res = run_bass_kernel_spmd(nc, in_maps, list(range(N_CORES)))
    nd2 = np.concatenate(
        [res.results[c]["v8o"] for c in range(N_CORES)], axis=0
    )  # [NQ, 8] = -d2, exact
    i8 = np.concatenate(
        [res.results[c]["i8o"] for c in range(N_CORES)], axis=0
    )  # [NQ, 8] candidate ref indices

    # ---- epilogue: identical (unjitted) jnp ops to the reference ----
    import jax.numpy as jnp

    d2c = np.maximum(-nd2, np.float32(0.0)).astype(np.float32)
    dist8 = np.asarray(jnp.sqrt(jnp.asarray(d2c)))  # device sqrt bits

    # rank candidates by (dist, ref index) — the reference's tie-break
    key = (dist8.view(np.uint32).astype(np.uint64) << np.uint64(14)) | \
        i8.astype(np.uint64)
    order = np.argsort(key, axis=1, kind="stable")[:, :K]
    knn_idx = np.take_along_axis(i8, order, axis=1).astype(np.int64)
    knn_dist = np.take_along_axis(dist8, order, axis=1)

    # weights + gather + weighted sum, exactly as the reference writes it
    dj = jnp.asarray(knn_dist)
    weights = 1.0 / (dj + EPS)
    weights = weights / jnp.sum(weights, axis=1, keepdims=True)
    knn_flow = jnp.asarray(f)[jnp.asarray(knn_idx)]
    out = jnp.sum(weights[..., None] * knn_flow, axis=1)
    return np.asarray(out)



# revision 2
# speedup vs baseline: 1.0034x; 1.0034x over previous
"""KNN flow accumulation (AccFlow) Trainium2 kernel — hybrid rewrite.

Problem: for each of Nq=16384 query points (3D), find k=3 nearest of
Nr=16384 ref points (Euclidean), take inverse-distance-weighted average
of the corresponding ref_flow vectors.

Sharding: queries split 2048/core across 8 NeuronCores; refs replicated.

Numerics: the reference (jax on the neuron backend) computes
    d2 = fl(fl(q2 + r2) - fl(2*(q@rT)));  dist = fl(sqrt(max(d2, 0)))
where q@rT is the PE fp32 matmul (4 bf16-ish passes, ~4e-7 rms noise on
d2).  The dataset is tightly clustered (median d3..d4 gap in d2 is
~2e-5, with a tail to 1e-8), so both the top-3 picks AND the 1/d weights
are determined by the exact bits of that noisy d2 — they must be
reproduced bit-for-bit.

Split of labor:
  - DEVICE (this kernel): a fast fp32 elementwise score
        s = fl(fl(fl(2qx*rx) + 2qy*ry) + 2qz*rz + fl(-r2 - q2))
    via a scalar_tensor_tensor chain (4 wide [128, 8192] ops per
    128-query block per ref-chunk) + DVE max/max_index for the sorted
    top-8 of each chunk (2 chunks) = 16 candidates per query.  The
    score's error vs the reference d2 is ~1e-6 absolute, while the
    13-gap margin inside the top-16 is >1e-4, so the reference's true
    top-3 is always among the 16 candidates.
  - HOST epilogue: bit-exact rescoring of the 16 candidates per query by
    emulating the PE 4-pass fp32 decomposition in numpy (H/L split to
    12-bit significands: products exact in fp32), then the identical
    fl(fl(q2+r2) - fl(2M)) arithmetic, device-bit sqrt via jnp, the
    reference's (dist, index) tie-break, and its exact weight/gather/sum
    sequence.

The execution stack is dispatch-bound (~5us fixed + ~6ns/elem per
instruction), so the kernel minimizes instruction count: 6 instructions
per (block, chunk) = 192 per core + ~6 DMAs, vs 672 for the matmul
formulation.
"""

import os
import sys

import numpy as np

for _p in ("/opt/trn_rl_repo", os.path.expanduser("~/.axon_site/_ro/trn_rl_repo")):
    if os.path.isdir(_p) and _p not in sys.path:
        sys.path.insert(0, _p)

import concourse.bacc as bacc
import concourse.mybir as mybir
from concourse.bass_utils import run_bass_kernel_spmd
from concourse.tile import TileContext

F32 = mybir.dt.float32
U32 = mybir.dt.uint32
MULT = mybir.AluOpType.mult
ADD = mybir.AluOpType.add

N_CORES = 8
NQ = 16384
NR = 16384
K = 3
EPS = 1e-8

P = 128                    # queries per block (partition dim)
NQ_CORE = NQ // N_CORES    # 2048
NB = NQ_CORE // P          # 16 blocks per core
NCH = 2                    # ref chunks
CH = NR // NCH             # 8192 refs per chunk

# engine assignment: "v" = vector, "g" = gpsimd
ENG_A = "v"                # the 3-op multiply-accumulate chain
ENG_B = "v"                # the final -(q2+r2) merge


def build_nc(reps=1):
    nc = bacc.Bacc(None, target_bir_lowering=False)

    # broadcast ref tables, identical on all 128 partitions:
    # [rx | ry | rz | -r2], each [128, NR]
    refs = nc.declare_dram_parameter("refs", [P, 4 * NR], F32, isOutput=False)
    # per-partition query scalars, block-major:
    # cols [c*NB + b] = 2*q_c[b*128+p] for c in 0..2; [3*NB+b] = -q2
    qs = nc.declare_dram_parameter("qs", [P, 4 * NB], F32, isOutput=False)
    i8o = nc.declare_dram_parameter("i8o", [NQ_CORE, 2 * 8], U32, isOutput=True)

    refs_v = refs[:, :].rearrange("p (c h x) -> p c h x", c=4, h=NCH)

    with TileContext(nc) as tc:
        with (
            tc.tile_pool(name="const", bufs=1) as const_pool,
            tc.tile_pool(name="ref", bufs=1) as ref_pool,
            tc.tile_pool(name="work", bufs=2) as work_pool,
            tc.tile_pool(name="outs", bufs=1) as out_pool,
        ):
            qs_sb = const_pool.tile([P, 4 * NB], F32, tag="qs")
            nc.sync.dma_start(out=qs_sb[:], in_=qs[:, :])
            vall = out_pool.tile([P, NB * 16], F32, tag="vall")
            iall = out_pool.tile([P, NB * 16], U32, tag="iall")

            eng_a = nc.vector if ENG_A == "v" else nc.gpsimd
            eng_b = nc.vector if ENG_B == "v" else nc.gpsimd

            for h in range(NCH):
                R = ref_pool.tile([P, 4 * CH], F32, tag="R")
                Rv = R[:].rearrange("p (c x) -> p c x", c=4)
                nc.sync.dma_start(out=Rv, in_=refs_v[:, :, h, :])
                rx = R[:, 0 * CH : 1 * CH]
                ry = R[:, 1 * CH : 2 * CH]
                rz = R[:, 2 * CH : 3 * CH]
                nr2 = R[:, 3 * CH : 4 * CH]

                for b in list(range(NB)) * reps:
                    qx = qs_sb[:, 0 * NB + b : 0 * NB + b + 1]
                    qy = qs_sb[:, 1 * NB + b : 1 * NB + b + 1]
                    qz = qs_sb[:, 2 * NB + b : 2 * NB + b + 1]
                    nq2 = qs_sb[:, 3 * NB + b : 3 * NB + b + 1]
                    p_t = work_pool.tile([P, CH], F32, tag="p")
                    # p = rx * 2qx ; p = ry*2qy + p ; p = rz*2qz + p
                    eng_a.tensor_scalar_mul(p_t[:], rx, qx)
                    eng_a.scalar_tensor_tensor(
                        out=p_t[:], in0=ry, scalar=qy, in1=p_t[:],
                        op0=MULT, op1=ADD,
                    )
                    eng_a.scalar_tensor_tensor(
                        out=p_t[:], in0=rz, scalar=qz, in1=p_t[:],
                        op0=MULT, op1=ADD,
                    )
                    # nd2 ~= fl( fl(-r2 + -q2) + 2qr )
                    eng_b.scalar_tensor_tensor(
                        out=p_t[:], in0=nr2, scalar=nq2, in1=p_t[:],
                        op0=ADD, op1=ADD,
                    )
                    vsl = vall[:, b * 16 + h * 8 : b * 16 + h * 8 + 8]
                    isl = iall[:, b * 16 + h * 8 : b * 16 + h * 8 + 8]
                    nc.vector.max(out=vsl, in_=p_t[:])
                    nc.vector.max_index(out=isl, in_max=vsl, in_values=p_t[:])

            # batched output: i8o[b*128+p, hj] = iall[p, b*16+hj]
            nc.sync.dma_start(
                out=i8o[:, :].rearrange("(b p) hj -> p b hj", p=P),
                in_=iall[:].rearrange("p (b hj) -> p b hj", hj=16),
            )

    nc.finalize()
    return nc


_NC_CACHE = None


def _get_nc():
    global _NC_CACHE
    if _NC_CACHE is None:
        _NC_CACHE = build_nc()
    return _NC_CACHE


def _emu_sumsq(x):
    """bitwise emulation of jnp.sum(x*x, axis=-1) in fp32: (x2+y2)+z2"""
    x = np.asarray(x, dtype=np.float32)
    x2 = x[:, 0] * x[:, 0]
    y2 = x[:, 1] * x[:, 1]
    z2 = x[:, 2] * x[:, 2]
    return (x2 + y2) + z2


def _trunc12(x):
    """truncate fp32 significand to 12 bits (PE e10m11 'H' part)"""
    u = np.ascontiguousarray(x, dtype=np.float32).view(np.uint32)
    return (u & np.uint32(0xFFFFF000)).view(np.float32)


def _pe_matmul_pairs(qv, rv):
    """Bit-exact emulation of the PE fp32 dot(q_i, r_j) over the last
    axis (3) for paired [..., 3] fp32 operands.

    Verified recipe (0 mismatches vs device on 2048x2048):
      H = trunc-to-12-bit-significand, L = exact residual
      pass(a,b) = fl(fl(a0*b0 + a1*b1) + a2*b2)   products exact
      M = fl( fl(HH + HL) + fl(LH + LL) )
    """
    qh = _trunc12(qv)
    ql = (qv - qh).astype(np.float32)
    rh = _trunc12(rv)
    rl = (rv - rh).astype(np.float32)

    def kchain(a, b):
        acc = (
            a[..., 0].astype(np.float64) * b[..., 0].astype(np.float64)
        ).astype(np.float32)
        for k in (1, 2):
            acc = (
                acc.astype(np.float64)
                + a[..., k].astype(np.float64) * b[..., k].astype(np.float64)
            ).astype(np.float32)
        return acc

    t1 = (kchain(qh, rh).astype(np.float64) + kchain(qh, rl)).astype(np.float32)
    t2 = (kchain(ql, rh).astype(np.float64) + kchain(ql, rl)).astype(np.float32)
    return (t1.astype(np.float64) + t2).astype(np.float32)


def prepare_in_maps(q, r):
    """Host-side tables for all 8 cores."""
    q = np.asarray(q, dtype=np.float32)
    r = np.asarray(r, dtype=np.float32)
    q2 = _emu_sumsq(q)
    r2 = _emu_sumsq(r)

    refs_row = np.empty((4, NR), dtype=np.float32)
    refs_row[:3] = r.T
    refs_row[3] = -r2
    refs = np.ascontiguousarray(
        np.broadcast_to(refs_row.reshape(1, 4 * NR), (P, 4 * NR))
    )

    in_maps = []
    for c in range(N_CORES):
        s = slice(c * NQ_CORE, (c + 1) * NQ_CORE)
        qsc = np.empty((4, NB, P), dtype=np.float32)
        qsc[:3] = (2.0 * q[s].T).reshape(3, NB, P)
        qsc[3] = (-q2[s]).reshape(NB, P)
        qs_t = np.ascontiguousarray(qsc.transpose(2, 0, 1).reshape(P, 4 * NB))
        in_maps.append({"refs": refs, "qs": qs_t})
    return in_maps


def kernel(query_points, ref_points, ref_flow, k):
    assert int(k) == K
    q = np.ascontiguousarray(np.asarray(query_points, dtype=np.float32))
    r = np.ascontiguousarray(np.asarray(ref_points, dtype=np.float32))
    f = np.ascontiguousarray(np.asarray(ref_flow, dtype=np.float32))
    assert q.shape == (NQ, 3) and r.shape == (NR, 3)

    nc = _get_nc()
    in_maps = prepare_in_maps(q, r)
    res = run_bass_kernel_spmd(nc, in_maps, list(range(N_CORES)))
    i8 = np.concatenate(
        [res.results[c]["i8o"] for c in range(N_CORES)], axis=0
    ).astype(np.int64)  # [NQ, 16] chunk-local candidate indices
    i8[:, 8:] += CH  # chunk-1 offset

    # ---- epilogue: bit-exact rescoring of the 16 candidates ----
    import jax.numpy as jnp

    q2 = _emu_sumsq(q)
    r2 = _emu_sumsq(r)
    qv = np.repeat(q[:, None, :], i8.shape[1], axis=1)  # [NQ, 16, 3]
    rv = r[i8]                                          # [NQ, 16, 3]
    M = _pe_matmul_pairs(qv, rv)                        # PE bits of q.r
    x = (q2[:, None] + r2[i8]).astype(np.float32)       # fl(q2+r2)
    d2 = (x - np.float32(2.0) * M).astype(np.float32)   # fl(X - 2M)
    d2c = np.maximum(d2, np.float32(0.0))
    dist8 = np.asarray(jnp.sqrt(jnp.asarray(d2c)))      # device sqrt bits

    # rank candidates by (dist, ref index) — the reference's tie-break
    key = (dist8.view(np.uint32).astype(np.uint64) << np.uint64(14)) | \
        i8.astype(np.uint64)
    order = np.argsort(key, axis=1, kind="stable")[:, :K]
    knn_idx = np.take_along_axis(i8, order, axis=1)
    knn_dist = np.take_along_axis(dist8, order, axis=1)

    # weights + gather + weighted sum, exactly as the reference writes it
    dj = jnp.asarray(knn_dist)
    weights = 1.0 / (dj + EPS)
    weights = weights / jnp.sum(weights, axis=1, keepdims=True)
    knn_flow = jnp.asarray(f)[jnp.asarray(knn_idx)]
    out = jnp.sum(weights[..., None] * knn_flow, axis=1)
    return np.asarray(out)
